# revision 57
# baseline (speedup 1.0000x reference)
"""Causal multi-head attention (B=4, L=2048, D=1024, H=16, HD=64) on 8 TRN2
NeuronCores.

Sharding: core c handles batch b = c//2 and head-group g = c%2 (8 heads =
512 output dims). Attention is fully independent per (b, h); no collectives.

v2 design (vs. v1's fp32r + transposed-PV):
  - All matmul operands bf16 (1 cycle/row on the PE at ANY output width;
    fp32r drops to 4 cycles/row below 256-wide outputs). PSUM accumulation
    stays fp32. Output stays fp32.
  - PV computes O directly: out[q,65] = es[:,qtile].T @ Vaug[m,65] per
    (head, q-tile-128, m-tile-128), streaming only 65 V columns per matmul.
    Halves PV cycles vs streaming q, and removes every PE transpose
    (the ones-column of Vaug still accumulates softmax denominators).
  - The QKV projections of chunk lc+1 are interleaved into attention(qc=lc)'s
    PE instruction stream: the Activation engine (exp) is the co-bottleneck,
    and the interleave keeps the PE busy while Act drains. Attention(qc=0)
    starts as soon as the needed Q/K dim-tiles of chunk 0 are done.
  - Engine placement: exp on Act; Q/K/V PSUM->SBUF bias-copies on DVE;
    causal-mask multiplies and the final denominator-scaling on GpSimd
    (otherwise idle); reciprocals on DVE.
"""

import sys

if "/opt/trn_rl_repo" not in sys.path:
    sys.path.insert(0, "/opt/trn_rl_repo")

import ml_dtypes
import numpy as np

import concourse.bass as bass  # noqa: F401
import concourse.bacc as bacc
import concourse.tile as tile
from concourse import mybir
from concourse.bass_utils import run_bass_kernel_spmd

B, L, D = 4, 2048, 1024
H, HD = 16, 64
NCORES = 8
DIMS = 512  # output dims per core (8 heads)
NKT = 8  # k-tiles over D
NDT = 4  # dim-tiles over DIMS
NQC = 4  # q-chunks of 512
NLT = 16  # l-tiles of 128
SCALE = 0.25  # 1/sqrt(H)
BF16 = mybir.dt.bfloat16
F32 = mybir.dt.float32
AF = mybir.ActivationFunctionType

_cache = {}


def _build_kernel(sps_bufs=2, es_bufs=34, qkv_bufs=2, po_bufs=1,
                  masks_on="gpsimd", fin_on="vector"):
    nc = bacc.Bacc("TRN2", target_bir_lowering=False, debug=False)

    # partition-major layouts so each logical tensor loads in ONE DMA
    # (per-DMA queue cost is ~0.6-1us; tile-by-tile loading delayed the
    # first matmul to ~16us).
    XT = nc.declare_dram_parameter("XT", [128, NKT, L], BF16, isOutput=False)
    WQT = nc.declare_dram_parameter("WQT", [128, NKT, DIMS], BF16, isOutput=False)
    WKT = nc.declare_dram_parameter("WKT", [128, NKT, DIMS], BF16, isOutput=False)
    WVT = nc.declare_dram_parameter("WVT", [128, NKT, DIMS], BF16, isOutput=False)
    BQ = nc.declare_dram_parameter("BQ", [NDT, 128, 1], F32, isOutput=False)
    BK = nc.declare_dram_parameter("BK", [NDT, 128, 1], F32, isOutput=False)
    BV = nc.declare_dram_parameter("BV", [1, DIMS], BF16, isOutput=False)
    MASKS = nc.declare_dram_parameter("MASKS", [128, 128], BF16, isOutput=False)
    OUT = nc.declare_dram_parameter("OUT", [NQC, 128, 4, DIMS], F32, isOutput=True)

    mask_eng = {"gpsimd": "gpsimd", "vector": "vector"}[masks_on]
    fin_eng = {"gpsimd": "gpsimd", "vector": "vector"}[fin_on]

    with tile.TileContext(nc) as tc:
        with tc.tile_pool(name="persist", bufs=1) as pp:
            # ---- resident tiles (const DMAs are issued AFTER x-chunk0 so
            # the sync queue serves the critical path first) ----
            mask0 = pp.tile([128, 128], BF16, tag="mask0", name="mask0")
            bq_sb = [pp.tile([128, 1], F32, tag=f"bq{d}", name=f"bq{d}") for d in range(NDT)]
            bk_sb = [pp.tile([128, 1], F32, tag=f"bk{d}", name=f"bk{d}") for d in range(NDT)]
            bv_sb = pp.tile([1, DIMS], BF16, tag="bv", name="bv")
            ones1 = pp.tile([1, 128], BF16, tag="ones1", name="ones1")
            nc.gpsimd.memset(ones1[0:1, :], 1.0)
            vb_sb = pp.tile([128, DIMS], BF16, tag="vbb", name="vbb")

            # QT/KT: [128 (2 heads), 2048 l] per dim-tile; Vaug: [128 l, 8, 65]
            qt = [pp.tile([128, L], BF16, tag=f"qt{d}", name=f"qt{d}") for d in range(NDT)]
            # K^T per head, zero-padded to K=128 so every attention matmul
            # keeps the same PE row config. Zero-fills are split DVE/GpSimd
            # and ordered by first-use deadline (S of hp=h needs ktp[2h,2h+1]).
            ktp = [pp.tile([128, L], BF16, tag=f"ktp{h}", name=f"ktp{h}") for h in range(8)]
            vaug = [pp.tile([128, 8, 65], BF16, tag=f"va{t}", name=f"va{t}") for t in range(NLT)]

            def _ktp_pad(h):
                return ktp[h][64:128, :] if h % 2 == 0 else ktp[h][0:64, :]

            for h in (0, 1):
                nc.vector.memset(_ktp_pad(h), 0.0)

            with (
                tc.tile_pool(name="wts", bufs=1) as wp,
                tc.tile_pool(name="xtp", bufs=16) as xtp,
                tc.tile_pool(name="psQ", bufs=qkv_bufs, space="PSUM") as psQ,
                tc.tile_pool(name="psS", bufs=sps_bufs, space="PSUM") as psS,
                tc.tile_pool(name="psPO", bufs=po_bufs, space="PSUM") as psPO,
                tc.tile_pool(name="esb", bufs=es_bufs) as esb,
                tc.tile_pool(name="fin", bufs=2) as fin,
            ):
                # Per-k tiles: multiple DMA writers to ONE tile serialize
                # (tile-granular writer ordering, ~5.7us per 128KB transfer),
                # so every 128KB transfer gets its own tile.  Queues: sync
                # carries x-chunk0 + consts + later chunks, scalar carries
                # wq + chunk1 + wv, gpsimd carries wk.
                wqt = [wp.tile([128, DIMS], BF16, tag=f"wq{k}", name=f"wq{k}") for k in range(NKT)]
                wkt = [wp.tile([128, DIMS], BF16, tag=f"wk{k}", name=f"wk{k}") for k in range(NKT)]
                wvt = [wp.tile([128, DIMS], BF16, tag=f"wv{k}", name=f"wv{k}") for k in range(NKT)]
                xts = {}

                def load_chunk(lc, eng):
                    ts = []
                    for k in range(NKT):
                        t = xtp.tile([128, 512], BF16, tag="xt", bufs=16, name="xt")
                        eng.dma_start(out=t, in_=XT[:, k, lc * 512 : (lc + 1) * 512])
                        ts.append(t)
                    xts[lc] = ts

                nc.sync.dma_start(out=mask0, in_=MASKS[:, :])
                # PE p-state warm-up: dummy matmuls from ~1.5us ramp the PE
                # to 2.4GHz before the real pipeline's first (serial) units.
                for _ in range(24):
                    wm = psQ.tile([128, 512], F32, tag="qkv", bufs=qkv_bufs, name="wm")
                    nc.tensor.matmul(wm[:, 0:128], mask0[:, :], mask0[:, :],
                                     start=True, stop=True)
                load_chunk(0, nc.sync)
                for k in range(NKT):
                    nc.scalar.dma_start(out=wqt[k], in_=WQT[:, k, :])
                for k in range(NKT):
                    nc.gpsimd.dma_start(out=wkt[k], in_=WKT[:, k, :])
                nc.sync.dma_start(out=bv_sb, in_=BV[:, :])
                # half of wv rides sync so the scalar (Act) queue drains
                # sooner: every DMA trigger there delays the first ACTIVATEs.
                for k in range(4, NKT):
                    nc.sync.dma_start(out=wvt[k], in_=WVT[:, k, :])
                for d in range(NDT):
                    nc.sync.dma_start(out=bq_sb[d], in_=BQ[d, :, :])
                    nc.sync.dma_start(out=bk_sb[d], in_=BK[d, :, :])
                load_chunk(1, nc.sync)
                for k in range(4):
                    nc.scalar.dma_start(out=wvt[k], in_=WVT[:, k, :])

                for h in (2, 3, 6, 7):
                    nc.gpsimd.memset(_ktp_pad(h), 0.0)
                for t in range(NLT):
                    nc.gpsimd.memset(vaug[t][:, :, 64:65], 1.0)

                def emit_vb():
                    # V bias broadcast to all 128 partitions: vb = ones^T @ bv
                    vb_ps = psQ.tile([128, DIMS], F32, tag="qkv", bufs=qkv_bufs, name="vbps")
                    nc.tensor.matmul(vb_ps, ones1[0:1, :], bv_sb[0:1, :], start=True, stop=True)
                    nc.vector.tensor_copy(vb_sb, vb_ps)

                def emit_ktp45():
                    for h in (4, 5):
                        nc.vector.memset(_ktp_pad(h), 0.0)

                # ---- QKV projection units (each: 8 PE matmuls + DVE copy) ----
                def emit_q(lc, d):
                    lsl = slice(lc * 512, (lc + 1) * 512)
                    dsl = slice(d * 128, (d + 1) * 128)
                    ps = psQ.tile([128, 512], F32, tag="qkv", bufs=qkv_bufs, name="psq")
                    for k in range(NKT):
                        nc.tensor.matmul(ps, wqt[k][:, dsl], xts[lc][k][:],
                                         start=(k == 0), stop=(k == NKT - 1))
                    nc.vector.tensor_scalar_add(qt[d][:, lsl], ps, bq_sb[d][:])

                def emit_k(lc, d):
                    lsl = slice(lc * 512, (lc + 1) * 512)
                    dsl = slice(d * 128, (d + 1) * 128)
                    ps = psQ.tile([128, 512], F32, tag="qkv", bufs=qkv_bufs, name="psk")
                    for k in range(NKT):
                        nc.tensor.matmul(ps, wkt[k][:, dsl], xts[lc][k][:],
                                         start=(k == 0), stop=(k == NKT - 1))
                    nc.vector.tensor_scalar_add(ktp[2 * d][0:64, lsl], ps[0:64, :], bk_sb[d][0:64])
                    nc.vector.tensor_scalar_add(ktp[2 * d + 1][64:128, lsl], ps[64:128, :], bk_sb[d][64:128])

                def emit_v(lc, lb):
                    lt = lc * 4 + lb
                    ps = psQ.tile([128, 512], F32, tag="qkv", bufs=qkv_bufs, name="psv")
                    for k in range(NKT):
                        nc.tensor.matmul(ps, xts[lc][k][:, lb * 128 : (lb + 1) * 128], wvt[k][:],
                                         start=(k == 0), stop=(k == NKT - 1))
                    nc.vector.tensor_add(
                        vaug[lt][:, :, 0:64],
                        ps[:].rearrange("p (h d) -> p h d", h=8),
                        vb_sb[:].rearrange("p (h d) -> p h d", h=8),
                    )

                # ---- attention scheduler ----
                # PSUM constraint: within one bank only ONE matmul accumulation
                # group may be open at a time (an open group's partial is
                # dropped when another region of the same bank starts).  So PV
                # groups are emitted as CONTIGUOUS per-bank runs, deferred by
                # one head-pair: while S/exp of (qc,hp) stream, the PV groups
                # of the previous head-pair (whose es tiles persist) are
                # emitted between the S matmuls, one open group per po bank.
                pv_queue = []  # deferred closures (PV groups / finalizes)
                feng = getattr(nc, fin_eng)
                meng = getattr(nc, mask_eng)

                def make_hp_pv(qc, hp, es_list, outb):
                    # [128,4,128] = exactly one 2KB PSUM bank per tile: the
                    # one-open-accumulation-group-per-bank invariant must not
                    # depend on allocator packing.
                    po_a = psPO.tile([128, 4, 128], F32, tag="poa", bufs=po_bufs, name="poa")
                    po_b = psPO.tile([128, 4, 128], F32, tag="pob", bufs=po_bufs, name="pob")

                    def group(qb, half, po):
                        def emit():
                            last = 4 * qc + qb
                            for mt in range(last + 1):
                                nc.tensor.matmul(
                                    po[:, qb, 0:65],
                                    es_list[mt][:, 512 * half + qb * 128 : 512 * half + (qb + 1) * 128],
                                    vaug[mt][:, 2 * hp + half, :],
                                    start=(mt == 0), stop=(mt == last))
                        return emit

                    groups = [(group(qb, 0, po_a), group(qb, 1, po_b))
                              for qb in range(4)]

                    def finalize():
                        for half, po in ((0, po_a), (1, po_b)):
                            h = 2 * hp + half
                            r = fin.tile([128, 4, 1], F32, tag="r", bufs=4, name="r")
                            nc.vector.reciprocal(r, po[:, :, 64:65])
                            for qb in range(4):
                                feng.tensor_scalar_mul(
                                    outb[:, qb, h * 64 : (h + 1) * 64],
                                    po[:, qb, 0:64], r[:, qb, :])
                        # staged output: the dim-slice a finalize completes is
                        # DMA'd immediately, so only heads 12-15 (64KB/qb)
                        # remain after the last finalize.
                        if hp == 1:
                            for qb in range(4):
                                nc.sync.dma_start(out=OUT[qc, :, qb, 0:256],
                                                  in_=outb[:, qb, 0:256])
                        elif hp == 2:
                            for qb in range(4):
                                nc.sync.dma_start(out=OUT[qc, :, qb, 256:384],
                                                  in_=outb[:, qb, 256:384])
                        elif hp == 3:
                            for qb, eng in enumerate((nc.sync, nc.scalar,
                                                      nc.gpsimd, nc.sync)):
                                eng.dma_start(out=OUT[qc, :, qb, 384:512],
                                              in_=outb[:, qb, 384:512])

                    def finalize_last():
                        # tail-optimized: each qb's output slice is DMA'd as
                        # soon as its two muls land.
                        ra = fin.tile([128, 4, 1], F32, tag="r", bufs=4, name="ra")
                        nc.vector.reciprocal(ra, po_a[:, :, 64:65])
                        rb = fin.tile([128, 4, 1], F32, tag="r", bufs=4, name="rb")
                        nc.vector.reciprocal(rb, po_b[:, :, 64:65])
                        engs = (nc.sync, nc.sync, nc.gpsimd, nc.sync)
                        for qb in range(4):
                            feng.tensor_scalar_mul(
                                outb[:, qb, 384:448], po_a[:, qb, 0:64], ra[:, qb, :])
                            feng.tensor_scalar_mul(
                                outb[:, qb, 448:512], po_b[:, qb, 0:64], rb[:, qb, :])
                            engs[qb].dma_start(out=OUT[qc, :, qb, 384:512],
                                               in_=outb[:, qb, 384:512])
                    return groups, (finalize_last if hp == 3 else finalize)

                def push_hp_pv(qc, hp, es_list, outb):
                    groups, finalize = make_hp_pv(qc, hp, es_list, outb)
                    for ga, gb in groups:
                        pv_queue.append(ga)
                        pv_queue.append(gb)
                    pv_queue.append(finalize)

                def emit_segment(qc, pre_units, slot_units, spread_units, final=False):
                    """pre_units: {hp: [unit,...]} emitted at that hp's start.
                    slot_units: {global_slot_idx: [unit,...]}.
                    spread_units: list spread evenly over all slots."""
                    nmt = 4 * qc + 4
                    total_slots = 4 * nmt
                    n_spread = len(spread_units)
                    spread_at = set()
                    if n_spread:
                        for i in range(n_spread):
                            spread_at.add(int((i + 0.5) * total_slots / n_spread))
                    spread_iter = iter(spread_units)
                    outb = fin.tile([128, 4, DIMS], F32, tag="outb", bufs=2, name="outb")
                    slot = 0
                    for hp in range(4):
                        for u in pre_units.get(hp, ()):
                            u()
                        # drain rate: finish the deferred queue within this
                        # block; mt==0 and mt==nmt-1 emit no pops so the
                        # first S matmuls of this block and the next go
                        # back-to-back and Act never starves at boundaries.
                        pops = (len(pv_queue) + nmt - 3) // max(nmt - 2, 1)
                        es_list = []
                        is_final = final and hp == 3
                        if is_final:
                            fgroups, ffinalize = make_hp_pv(qc, hp, es_list, outb)
                        for mt in range(nmt):
                            msl = slice(mt * 128, (mt + 1) * 128)
                            off = mt * 128 - qc * 512
                            o = max(0, off)
                            qa = slice(qc * 512 + o, (qc + 1) * 512)
                            s_ps = psS.tile([128, 1024], F32, tag="sps", bufs=sps_bufs, name="sps")
                            nc.tensor.matmul(s_ps[:, o:512], ktp[2 * hp][:, msl],
                                             qt[hp][:, qa], start=True, stop=True)
                            nc.tensor.matmul(s_ps[:, 512 + o : 1024], ktp[2 * hp + 1][:, msl],
                                             qt[hp][:, qa], start=True, stop=True)
                            es = esb.tile([128, 1024], BF16, tag="es", bufs=es_bufs, name="es")
                            if o <= 128:
                                nc.scalar.activation(es[:, o:1024], s_ps[:, o:1024], AF.Exp, scale=SCALE)
                            else:
                                nc.scalar.activation(es[:, o:512], s_ps[:, o:512], AF.Exp, scale=SCALE)
                                nc.scalar.activation(es[:, 512 + o : 1024], s_ps[:, 512 + o : 1024],
                                                     AF.Exp, scale=SCALE)
                            if off >= 0:  # triangular 128-col edge of the block
                                meng.tensor_mul(es[:, o : o + 128], es[:, o : o + 128], mask0[:, :])
                                meng.tensor_mul(es[:, 512 + o : 512 + o + 128],
                                                es[:, 512 + o : 512 + o + 128], mask0[:, :])
                            es_list.append(es)
                            for u in slot_units.get(slot, ()):
                                u()
                            if slot in spread_at:
                                u = next(spread_iter, None)
                                if u is not None:
                                    u()
                            slot += 1
                            if 0 < mt < nmt - 1 or (final and hp == 3):
                                for _ in range(pops):
                                    if pv_queue:
                                        pv_queue.pop(0)()
                            # final block: emit PV groups as soon as the exps
                            # they need are in flight, shrinking the drain tail
                            if is_final and mt >= nmt - 3 and mt < nmt - 1:
                                qb = mt - (nmt - 3)
                                fgroups[qb][0]()
                                fgroups[qb][1]()
                        if is_final:
                            for qb in (2, 3):
                                fgroups[qb][0]()
                                fgroups[qb][1]()
                            ffinalize()
                        else:
                            push_hp_pv(qc, hp, es_list, outb)

                # ---- schedule ----
                # seg qc=0: chunk-0 Q/K for hp=0 as pre-units; hp 1..3's Q/K
                # prefetched one head-pair early via slots; V0 and chunk-1
                # units at explicit slots (deadline: before seg qc=1).
                q0 = [lambda d=d: emit_q(0, d) for d in range(NDT)]
                k0 = [lambda d=d: emit_k(0, d) for d in range(NDT)]
                v0 = [lambda b=b: emit_v(0, b) for b in range(4)]
                q1 = [lambda d=d: emit_q(1, d) for d in range(NDT)]
                k1 = [lambda d=d: emit_k(1, d) for d in range(NDT)]
                v1 = [lambda b=b: emit_v(1, b) for b in range(4)]
                emit_segment(
                    0,
                    pre_units={0: [q0[0], k0[0], emit_vb]},
                    slot_units={
                        1: [q0[1], k0[1]], 2: [emit_ktp45, v0[0]],
                        3: [v0[1]], 5: [v0[2], q0[2]], 6: [k0[2]],
                        7: [v0[3]], 9: [q1[0], q0[3]], 10: [k0[3]],
                        11: [q1[1], q1[2]], 13: [q1[3], k1[0]],
                        14: [k1[1], k1[2], k1[3]],
                        15: [v1[0], v1[1], v1[2], v1[3]],
                    },
                    spread_units=[],
                )
                # Late segments are Act(exp)-bound, early ones PE-bound, so
                # projection units migrate as late as their first use allows:
                # K2[d] feeds S(qc2,hp=d,mt=8) at seg-qc2 slot 16d+8; Q2[d]
                # feeds S(qc2,hp=d,mt=0) at slot 16d; V2 feeds PV pops of
                # (qc2,hp0) during hp1's block (slot>=16); Q3[d] feeds
                # S(qc3,hp=d,mt=0); K3[d] feeds S(qc3,hp=d,mt=12); V3 feeds
                # PV pops of (qc3,hp0) at slot>=16.
                q2 = [lambda d=d: emit_q(2, d) for d in range(NDT)]
                k2 = [lambda d=d: emit_k(2, d) for d in range(NDT)]
                v2 = [lambda b=b: emit_v(2, b) for b in range(4)]
                q3 = [lambda d=d: emit_q(3, d) for d in range(NDT)]
                k3 = [lambda d=d: emit_k(3, d) for d in range(NDT)]
                v3 = [lambda b=b: emit_v(3, b) for b in range(4)]
                load_chunk(2, nc.sync)
                emit_segment(
                    1, pre_units={},
                    slot_units={10: [q2[0]], 20: [k2[0]]},
                    spread_units=[],
                )
                load_chunk(3, nc.sync)
                emit_segment(
                    2, pre_units={},
                    slot_units={1: [q2[1], v2[0]], 2: [v2[1]], 4: [v2[2]],
                                6: [v2[3]], 8: [k2[1]], 13: [q2[2]],
                                20: [k2[2]], 26: [q2[3]], 37: [k2[3]],
                                42: [q3[0]]},
                    spread_units=[],
                )
                emit_segment(
                    3, pre_units={},
                    slot_units={2: [k3[0]], 4: [v3[0]], 6: [v3[1]],
                                8: [v3[2]], 10: [v3[3]], 13: [q3[1]],
                                20: [k3[1]], 24: [q3[2]], 36: [k3[2]],
                                40: [q3[3]], 52: [k3[3]]},
                    spread_units=[], final=True,
                )
                # drain the deferred PV work of the last head pair
                for u in pv_queue:
                    u()
                pv_queue.clear()

    nc.compile()
    return nc


def _host_inputs(X, Wq, bq, Wk, bk, Wv, bv):
    """Build the 8 per-core input maps (host-side sharding + layout prep)."""
    X = np.asarray(X, dtype=np.float32)
    Wq = np.asarray(Wq, dtype=np.float32)
    Wk = np.asarray(Wk, dtype=np.float32)
    Wv = np.asarray(Wv, dtype=np.float32)
    bq = np.asarray(bq, dtype=np.float32)
    bk = np.asarray(bk, dtype=np.float32)
    bv = np.asarray(bv, dtype=np.float32)

    bf = ml_dtypes.bfloat16
    mask = (np.arange(128)[None, :] >= np.arange(128)[:, None]).astype(bf)

    def pmaj(a):  # [D, n] -> [128, NKT, n] partition-major
        return np.ascontiguousarray(a.reshape(NKT, 128, -1).transpose(1, 0, 2))

    in_maps = []
    for c in range(NCORES):
        b, g = divmod(c, 2)
        dsl = slice(g * DIMS, (g + 1) * DIMS)
        in_maps.append(
            {
                "XT": pmaj(np.ascontiguousarray(X[b].T)).astype(bf),
                "WQT": pmaj(np.ascontiguousarray(Wq[dsl, :].T)).astype(bf),
                "WKT": pmaj(np.ascontiguousarray(Wk[dsl, :].T)).astype(bf),
                "WVT": pmaj(np.ascontiguousarray(Wv[dsl, :].T)).astype(bf),
                "BQ": np.ascontiguousarray(bq[dsl].reshape(NDT, 128, 1)),
                "BK": np.ascontiguousarray(bk[dsl].reshape(NDT, 128, 1)),
                "BV": np.ascontiguousarray(bv[dsl].reshape(1, DIMS)).astype(bf),
                "MASKS": mask,
            }
        )
    return in_maps


def _run(in_maps, trace=False, variant=None):
    key = ("nc", variant)
    if key not in _cache:
        kw = dict(VARIANTS.get(variant, {}))
        _cache[key] = _build_kernel(**kw)
    res = run_bass_kernel_spmd(
        _cache[key], in_maps, core_ids=list(range(NCORES)), trace=trace
    )
    return res


VARIANTS = {
    None: {},
    "sps3": {"sps_bufs": 3},
    "po2": {"po_bufs": 2},
    "maskdve": {"masks_on": "vector"},
}


def _assemble(res):
    out = np.empty((B, L, D), dtype=np.float32)
    for c in range(NCORES):
        b, g = divmod(c, 2)
        o = res.results[c]["OUT"]  # [qc, 128, qb, 512]
        out[b, :, g * DIMS : (g + 1) * DIMS] = (
            o.transpose(0, 2, 1, 3).reshape(L, DIMS)
        )
    return out


def kernel(X, Wq, bq, Wk, bk, Wv, bv):
    in_maps = _host_inputs(X, Wq, bq, Wk, bk, Wv, bv)
    res = _run(in_maps, trace=False)
    return _assemble(res)


# revision 60
# speedup vs baseline: 1.2007x; 1.2007x over previous
"""Causal multi-head attention (B=4, L=2048, D=1024, H=16, HD=64) on 8 TRN2
NeuronCores.

Sharding: core c handles batch b = c//2 and head-group g = c%2 (8 heads =
512 output dims). Attention is fully independent per (b, h); no collectives.

v2 design (vs. v1's fp32r + transposed-PV):
  - All matmul operands bf16 (1 cycle/row on the PE at ANY output width;
    fp32r drops to 4 cycles/row below 256-wide outputs). PSUM accumulation
    stays fp32. Output stays fp32.
  - PV computes O directly: out[q,65] = es[:,qtile].T @ Vaug[m,65] per
    (head, q-tile-128, m-tile-128), streaming only 65 V columns per matmul.
    Halves PV cycles vs streaming q, and removes every PE transpose
    (the ones-column of Vaug still accumulates softmax denominators).
  - The QKV projections of chunk lc+1 are interleaved into attention(qc=lc)'s
    PE instruction stream: the Activation engine (exp) is the co-bottleneck,
    and the interleave keeps the PE busy while Act drains. Attention(qc=0)
    starts as soon as the needed Q/K dim-tiles of chunk 0 are done.
  - Engine placement: exp on Act; Q/K/V PSUM->SBUF bias-copies on DVE;
    causal-mask multiplies and the final denominator-scaling on GpSimd
    (otherwise idle); reciprocals on DVE.
"""

import sys

if "/opt/trn_rl_repo" not in sys.path:
    sys.path.insert(0, "/opt/trn_rl_repo")

import ml_dtypes
import numpy as np

import concourse.bass as bass  # noqa: F401
import concourse.bacc as bacc
import concourse.tile as tile
from concourse import mybir
from concourse.bass_utils import run_bass_kernel_spmd

B, L, D = 4, 2048, 1024
H, HD = 16, 64
NCORES = 8
DIMS = 512  # output dims per core (8 heads)
NKT = 8  # k-tiles over D
NDT = 4  # dim-tiles over DIMS
NQC = 4  # q-chunks of 512
NLT = 16  # l-tiles of 128
SCALE = 0.25  # 1/sqrt(H)
BF16 = mybir.dt.bfloat16
F32 = mybir.dt.float32
AF = mybir.ActivationFunctionType

_cache = {}


def _build_kernel(sps_bufs=2, es_bufs=34, qkv_bufs=2, po_bufs=1,
                  masks_on="gpsimd", fin_on="vector"):
    nc = bacc.Bacc("TRN2", target_bir_lowering=False, debug=False)

    # partition-major layouts so each logical tensor loads in ONE DMA
    # (per-DMA queue cost is ~0.6-1us; tile-by-tile loading delayed the
    # first matmul to ~16us).
    XT = nc.declare_dram_parameter("XT", [128, NKT, L], BF16, isOutput=False)
    WQT = nc.declare_dram_parameter("WQT", [128, NKT, DIMS], BF16, isOutput=False)
    WKT = nc.declare_dram_parameter("WKT", [128, NKT, DIMS], BF16, isOutput=False)
    WVT = nc.declare_dram_parameter("WVT", [128, NKT, DIMS], BF16, isOutput=False)
    BQ = nc.declare_dram_parameter("BQ", [NDT, 128, 1], F32, isOutput=False)
    BK = nc.declare_dram_parameter("BK", [NDT, 128, 1], F32, isOutput=False)
    BV = nc.declare_dram_parameter("BV", [1, DIMS], BF16, isOutput=False)
    MASKS = nc.declare_dram_parameter("MASKS", [128, 128], BF16, isOutput=False)
    OUT = nc.declare_dram_parameter("OUT", [NQC, 128, 4, DIMS], F32, isOutput=True)

    mask_eng = {"gpsimd": "gpsimd", "vector": "vector"}[masks_on]
    fin_eng = {"gpsimd": "gpsimd", "vector": "vector"}[fin_on]

    with tile.TileContext(nc) as tc:
        with tc.tile_pool(name="persist", bufs=1) as pp:
            # ---- resident tiles (const DMAs are issued AFTER x-chunk0 so
            # the sync queue serves the critical path first) ----
            mask0 = pp.tile([128, 128], BF16, tag="mask0", name="mask0")
            bq_sb = [pp.tile([128, 1], F32, tag=f"bq{d}", name=f"bq{d}") for d in range(NDT)]
            bk_sb = [pp.tile([128, 1], F32, tag=f"bk{d}", name=f"bk{d}") for d in range(NDT)]
            bv_sb = pp.tile([1, DIMS], BF16, tag="bv", name="bv")
            ones1 = pp.tile([1, 128], BF16, tag="ones1", name="ones1")
            nc.gpsimd.memset(ones1[0:1, :], 1.0)
            vb_sb = pp.tile([128, DIMS], BF16, tag="vbb", name="vbb")

            # QT/KT: [128 (2 heads), 2048 l] per dim-tile; Vaug: [128 l, 8, 65]
            qt = [pp.tile([128, L], BF16, tag=f"qt{d}", name=f"qt{d}") for d in range(NDT)]
            # K^T per head, zero-padded to K=128 so every attention matmul
            # keeps the same PE row config. Zero-fills are split DVE/GpSimd
            # and ordered by first-use deadline (S of hp=h needs ktp[2h,2h+1]).
            ktp = [pp.tile([128, L], BF16, tag=f"ktp{h}", name=f"ktp{h}") for h in range(8)]
            vaug = [pp.tile([128, 8, 65], BF16, tag=f"va{t}", name=f"va{t}") for t in range(NLT)]

            def _ktp_pad(h):
                return ktp[h][64:128, :] if h % 2 == 0 else ktp[h][0:64, :]

            for h in (0, 1):
                nc.vector.memset(_ktp_pad(h), 0.0)

            with (
                tc.tile_pool(name="wts", bufs=1) as wp,
                tc.tile_pool(name="xtp", bufs=16) as xtp,
                tc.tile_pool(name="psQ", bufs=qkv_bufs, space="PSUM") as psQ,
                tc.tile_pool(name="psS", bufs=sps_bufs, space="PSUM") as psS,
                tc.tile_pool(name="psPO", bufs=po_bufs, space="PSUM") as psPO,
                tc.tile_pool(name="esb", bufs=es_bufs) as esb,
                tc.tile_pool(name="fin", bufs=2) as fin,
            ):
                # Per-k tiles: multiple DMA writers to ONE tile serialize
                # (tile-granular writer ordering, ~5.7us per 128KB transfer),
                # so every 128KB transfer gets its own tile.  Queues: sync
                # carries x-chunk0 + consts + later chunks, scalar carries
                # wq + chunk1 + wv, gpsimd carries wk.
                wqt = [wp.tile([128, DIMS], BF16, tag=f"wq{k}", name=f"wq{k}") for k in range(NKT)]
                wkt = [wp.tile([128, DIMS], BF16, tag=f"wk{k}", name=f"wk{k}") for k in range(NKT)]
                wvt = [wp.tile([128, DIMS], BF16, tag=f"wv{k}", name=f"wv{k}") for k in range(NKT)]
                xts = {}

                def load_chunk(lc, eng):
                    ts = []
                    for k in range(NKT):
                        t = xtp.tile([128, 512], BF16, tag="xt", bufs=16, name="xt")
                        eng.dma_start(out=t, in_=XT[:, k, lc * 512 : (lc + 1) * 512])
                        ts.append(t)
                    xts[lc] = ts

                nc.sync.dma_start(out=mask0, in_=MASKS[:, :])
                # PE p-state warm-up: dummy matmuls from ~1.5us ramp the PE
                # to 2.4GHz before the real pipeline's first (serial) units.
                for _ in range(24):
                    wm = psQ.tile([128, 512], F32, tag="qkv", bufs=qkv_bufs, name="wm")
                    nc.tensor.matmul(wm[:, 0:128], mask0[:, :], mask0[:, :],
                                     start=True, stop=True)
                load_chunk(0, nc.sync)
                for k in range(NKT):
                    nc.scalar.dma_start(out=wqt[k], in_=WQT[:, k, :])
                for k in range(NKT):
                    nc.gpsimd.dma_start(out=wkt[k], in_=WKT[:, k, :])
                nc.sync.dma_start(out=bv_sb, in_=BV[:, :])
                for d in range(NDT):
                    nc.sync.dma_start(out=bq_sb[d], in_=BQ[d, :, :])
                    nc.sync.dma_start(out=bk_sb[d], in_=BK[d, :, :])
                # chunk 1 rides the sync queue: every DMA trigger on the
                # scalar queue delays the first ACTIVATEs (in-order issue).
                load_chunk(1, nc.sync)
                for k in range(NKT):
                    nc.scalar.dma_start(out=wvt[k], in_=WVT[:, k, :])

                for h in (2, 3, 6, 7):
                    nc.gpsimd.memset(_ktp_pad(h), 0.0)
                for t in range(NLT):
                    nc.gpsimd.memset(vaug[t][:, :, 64:65], 1.0)

                def emit_vb():
                    # V bias broadcast to all 128 partitions: vb = ones^T @ bv
                    vb_ps = psQ.tile([128, DIMS], F32, tag="qkv", bufs=qkv_bufs, name="vbps")
                    nc.tensor.matmul(vb_ps, ones1[0:1, :], bv_sb[0:1, :], start=True, stop=True)
                    nc.vector.tensor_copy(vb_sb, vb_ps)

                def emit_ktp45():
                    for h in (4, 5):
                        nc.vector.memset(_ktp_pad(h), 0.0)

                # ---- QKV projection units (each: 8 PE matmuls + DVE copy) ----
                def emit_q(lc, d):
                    lsl = slice(lc * 512, (lc + 1) * 512)
                    dsl = slice(d * 128, (d + 1) * 128)
                    ps = psQ.tile([128, 512], F32, tag="qkv", bufs=qkv_bufs, name="psq")
                    for k in range(NKT):
                        nc.tensor.matmul(ps, wqt[k][:, dsl], xts[lc][k][:],
                                         start=(k == 0), stop=(k == NKT - 1))
                    nc.vector.tensor_scalar_add(qt[d][:, lsl], ps, bq_sb[d][:])

                def emit_k(lc, d):
                    lsl = slice(lc * 512, (lc + 1) * 512)
                    dsl = slice(d * 128, (d + 1) * 128)
                    ps = psQ.tile([128, 512], F32, tag="qkv", bufs=qkv_bufs, name="psk")
                    for k in range(NKT):
                        nc.tensor.matmul(ps, wkt[k][:, dsl], xts[lc][k][:],
                                         start=(k == 0), stop=(k == NKT - 1))
                    nc.vector.tensor_scalar_add(ktp[2 * d][0:64, lsl], ps[0:64, :], bk_sb[d][0:64])
                    nc.vector.tensor_scalar_add(ktp[2 * d + 1][64:128, lsl], ps[64:128, :], bk_sb[d][64:128])

                def emit_v(lc, lb):
                    lt = lc * 4 + lb
                    ps = psQ.tile([128, 512], F32, tag="qkv", bufs=qkv_bufs, name="psv")
                    for k in range(NKT):
                        nc.tensor.matmul(ps, xts[lc][k][:, lb * 128 : (lb + 1) * 128], wvt[k][:],
                                         start=(k == 0), stop=(k == NKT - 1))
                    nc.vector.tensor_add(
                        vaug[lt][:, :, 0:64],
                        ps[:].rearrange("p (h d) -> p h d", h=8),
                        vb_sb[:].rearrange("p (h d) -> p h d", h=8),
                    )

                # ---- attention scheduler ----
                # PSUM constraint: within one bank only ONE matmul accumulation
                # group may be open at a time (an open group's partial is
                # dropped when another region of the same bank starts).  So PV
                # groups are emitted as CONTIGUOUS per-bank runs, deferred by
                # one head-pair: while S/exp of (qc,hp) stream, the PV groups
                # of the previous head-pair (whose es tiles persist) are
                # emitted between the S matmuls, one open group per po bank.
                pv_queue = []  # deferred closures (PV groups / finalizes)
                feng = getattr(nc, fin_eng)
                meng = getattr(nc, mask_eng)

                def make_hp_pv(qc, hp, es_list, outb):
                    # [128,4,128] = exactly one 2KB PSUM bank per tile: the
                    # one-open-accumulation-group-per-bank invariant must not
                    # depend on allocator packing.
                    po_a = psPO.tile([128, 4, 128], F32, tag="poa", bufs=po_bufs, name="poa")
                    po_b = psPO.tile([128, 4, 128], F32, tag="pob", bufs=po_bufs, name="pob")

                    def group(qb, half, po):
                        def emit():
                            last = 4 * qc + qb
                            for mt in range(last + 1):
                                nc.tensor.matmul(
                                    po[:, qb, 0:65],
                                    es_list[mt][:, 512 * half + qb * 128 : 512 * half + (qb + 1) * 128],
                                    vaug[mt][:, 2 * hp + half, :],
                                    start=(mt == 0), stop=(mt == last))
                        return emit

                    groups = [(group(qb, 0, po_a), group(qb, 1, po_b))
                              for qb in range(4)]

                    def finalize():
                        for half, po in ((0, po_a), (1, po_b)):
                            h = 2 * hp + half
                            r = fin.tile([128, 4, 1], F32, tag="r", bufs=4, name="r")
                            nc.vector.reciprocal(r, po[:, :, 64:65])
                            for qb in range(4):
                                feng.tensor_scalar_mul(
                                    outb[:, qb, h * 64 : (h + 1) * 64],
                                    po[:, qb, 0:64], r[:, qb, :])
                        # staged output: the dim-slice a finalize completes is
                        # DMA'd immediately, so only heads 12-15 (64KB/qb)
                        # remain after the last finalize.
                        if hp == 1:
                            for qb in range(4):
                                nc.sync.dma_start(out=OUT[qc, :, qb, 0:256],
                                                  in_=outb[:, qb, 0:256])
                        elif hp == 2:
                            for qb in range(4):
                                nc.sync.dma_start(out=OUT[qc, :, qb, 256:384],
                                                  in_=outb[:, qb, 256:384])
                        elif hp == 3:
                            for qb, eng in enumerate((nc.sync, nc.scalar,
                                                      nc.gpsimd, nc.sync)):
                                eng.dma_start(out=OUT[qc, :, qb, 384:512],
                                              in_=outb[:, qb, 384:512])

                    def finalize_last():
                        # tail-optimized: each qb's output slice is DMA'd as
                        # soon as its two muls land.
                        ra = fin.tile([128, 4, 1], F32, tag="r", bufs=4, name="ra")
                        nc.vector.reciprocal(ra, po_a[:, :, 64:65])
                        rb = fin.tile([128, 4, 1], F32, tag="r", bufs=4, name="rb")
                        nc.vector.reciprocal(rb, po_b[:, :, 64:65])
                        # no scalar-queue DMA here: it would sit ahead of
                        # later ACTIVATEs in the Act sequencer's order.
                        engs = (nc.sync, nc.sync, nc.gpsimd, nc.sync)
                        for qb in range(4):
                            feng.tensor_scalar_mul(
                                outb[:, qb, 384:448], po_a[:, qb, 0:64], ra[:, qb, :])
                            feng.tensor_scalar_mul(
                                outb[:, qb, 448:512], po_b[:, qb, 0:64], rb[:, qb, :])
                            engs[qb].dma_start(out=OUT[qc, :, qb, 384:512],
                                               in_=outb[:, qb, 384:512])
                    return groups, (finalize_last if hp == 3 else finalize)

                def push_hp_pv(qc, hp, es_list, outb):
                    groups, finalize = make_hp_pv(qc, hp, es_list, outb)
                    for ga, gb in groups:
                        pv_queue.append(ga)
                        pv_queue.append(gb)
                    pv_queue.append(finalize)

                def emit_segment(qc, pre_units, slot_units, spread_units, final=False):
                    """pre_units: {hp: [unit,...]} emitted at that hp's start.
                    slot_units: {global_slot_idx: [unit,...]}.
                    spread_units: list spread evenly over all slots."""
                    nmt = 4 * qc + 4
                    total_slots = 4 * nmt
                    n_spread = len(spread_units)
                    spread_at = set()
                    if n_spread:
                        for i in range(n_spread):
                            spread_at.add(int((i + 0.5) * total_slots / n_spread))
                    spread_iter = iter(spread_units)
                    outb = fin.tile([128, 4, DIMS], F32, tag="outb", bufs=2, name="outb")
                    slot = 0
                    for hp in range(4):
                        for u in pre_units.get(hp, ()):
                            u()
                        # drain rate: finish the deferred queue within this
                        # block; mt==0 and mt==nmt-1 emit no pops so the
                        # first S matmuls of this block and the next go
                        # back-to-back and Act never starves at boundaries.
                        pops = (len(pv_queue) + nmt - 3) // max(nmt - 2, 1)
                        es_list = []
                        is_final = final and hp == 3
                        if is_final:
                            fgroups, ffinalize = make_hp_pv(qc, hp, es_list, outb)
                        for mt in range(nmt):
                            msl = slice(mt * 128, (mt + 1) * 128)
                            off = mt * 128 - qc * 512
                            o = max(0, off)
                            qa = slice(qc * 512 + o, (qc + 1) * 512)
                            s_ps = psS.tile([128, 1024], F32, tag="sps", bufs=sps_bufs, name="sps")
                            nc.tensor.matmul(s_ps[:, o:512], ktp[2 * hp][:, msl],
                                             qt[hp][:, qa], start=True, stop=True)
                            nc.tensor.matmul(s_ps[:, 512 + o : 1024], ktp[2 * hp + 1][:, msl],
                                             qt[hp][:, qa], start=True, stop=True)
                            es = esb.tile([128, 1024], BF16, tag="es", bufs=es_bufs, name="es")
                            if o <= 128:
                                nc.scalar.activation(es[:, o:1024], s_ps[:, o:1024], AF.Exp, scale=SCALE)
                            else:
                                nc.scalar.activation(es[:, o:512], s_ps[:, o:512], AF.Exp, scale=SCALE)
                                nc.scalar.activation(es[:, 512 + o : 1024], s_ps[:, 512 + o : 1024],
                                                     AF.Exp, scale=SCALE)
                            if off >= 0:  # triangular 128-col edge of the block
                                meng.tensor_mul(es[:, o : o + 128], es[:, o : o + 128], mask0[:, :])
                                meng.tensor_mul(es[:, 512 + o : 512 + o + 128],
                                                es[:, 512 + o : 512 + o + 128], mask0[:, :])
                            es_list.append(es)
                            for u in slot_units.get(slot, ()):
                                u()
                            if slot in spread_at:
                                u = next(spread_iter, None)
                                if u is not None:
                                    u()
                            slot += 1
                            if 0 < mt < nmt - 1 or (final and hp == 3):
                                for _ in range(pops):
                                    if pv_queue:
                                        pv_queue.pop(0)()
                            # final block: emit PV groups as soon as the exps
                            # they need are in flight, shrinking the drain tail
                            if is_final and mt >= nmt - 3 and mt < nmt - 1:
                                qb = mt - (nmt - 3)
                                fgroups[qb][0]()
                                fgroups[qb][1]()
                        if is_final:
                            for qb in (2, 3):
                                fgroups[qb][0]()
                                fgroups[qb][1]()
                            ffinalize()
                        else:
                            push_hp_pv(qc, hp, es_list, outb)

                # ---- schedule ----
                # seg qc=0: chunk-0 Q/K for hp=0 as pre-units; hp 1..3's Q/K
                # prefetched one head-pair early via slots; V0 and chunk-1
                # units at explicit slots (deadline: before seg qc=1).
                q0 = [lambda d=d: emit_q(0, d) for d in range(NDT)]
                k0 = [lambda d=d: emit_k(0, d) for d in range(NDT)]
                v0 = [lambda b=b: emit_v(0, b) for b in range(4)]
                q1 = [lambda d=d: emit_q(1, d) for d in range(NDT)]
                k1 = [lambda d=d: emit_k(1, d) for d in range(NDT)]
                v1 = [lambda b=b: emit_v(1, b) for b in range(4)]
                emit_segment(
                    0,
                    pre_units={0: [q0[0], k0[0], emit_vb]},
                    slot_units={
                        1: [q0[1], k0[1]], 2: [emit_ktp45, v0[0]],
                        3: [v0[1]], 5: [v0[2], q0[2]], 6: [k0[2]],
                        7: [v0[3]], 9: [q1[0], q0[3]], 10: [k0[3]],
                        11: [q1[1], q1[2]], 13: [q1[3], k1[0]],
                        14: [k1[1], k1[2], k1[3]],
                        15: [v1[0], v1[1], v1[2], v1[3]],
                    },
                    spread_units=[],
                )
                # Late segments are Act(exp)-bound, early ones PE-bound, so
                # projection units migrate as late as their first use allows:
                # K2[d] feeds S(qc2,hp=d,mt=8) at seg-qc2 slot 16d+8; Q2[d]
                # feeds S(qc2,hp=d,mt=0) at slot 16d; V2 feeds PV pops of
                # (qc2,hp0) during hp1's block (slot>=16); Q3[d] feeds
                # S(qc3,hp=d,mt=0); K3[d] feeds S(qc3,hp=d,mt=12); V3 feeds
                # PV pops of (qc3,hp0) at slot>=16.
                q2 = [lambda d=d: emit_q(2, d) for d in range(NDT)]
                k2 = [lambda d=d: emit_k(2, d) for d in range(NDT)]
                v2 = [lambda b=b: emit_v(2, b) for b in range(4)]
                q3 = [lambda d=d: emit_q(3, d) for d in range(NDT)]
                k3 = [lambda d=d: emit_k(3, d) for d in range(NDT)]
                v3 = [lambda b=b: emit_v(3, b) for b in range(4)]
                load_chunk(2, nc.sync)
                emit_segment(
                    1, pre_units={},
                    slot_units={10: [q2[0]], 20: [k2[0]]},
                    spread_units=[],
                )
                load_chunk(3, nc.sync)
                emit_segment(
                    2, pre_units={},
                    slot_units={1: [q2[1], v2[0]], 2: [v2[1]], 4: [v2[2]],
                                6: [v2[3]], 8: [k2[1]], 13: [q2[2]],
                                20: [k2[2]], 26: [q2[3]], 37: [k2[3]],
                                42: [q3[0]]},
                    spread_units=[],
                )
                emit_segment(
                    3, pre_units={},
                    slot_units={2: [k3[0]], 4: [v3[0]], 6: [v3[1]],
                                8: [v3[2]], 10: [v3[3]], 13: [q3[1]],
                                20: [k3[1]], 24: [q3[2]], 36: [k3[2]],
                                40: [q3[3]], 52: [k3[3]]},
                    spread_units=[], final=True,
                )
                # drain the deferred PV work of the last head pair
                for u in pv_queue:
                    u()
                pv_queue.clear()

    nc.compile()
    return nc


def _host_inputs(X, Wq, bq, Wk, bk, Wv, bv):
    """Build the 8 per-core input maps (host-side sharding + layout prep)."""
    X = np.asarray(X, dtype=np.float32)
    Wq = np.asarray(Wq, dtype=np.float32)
    Wk = np.asarray(Wk, dtype=np.float32)
    Wv = np.asarray(Wv, dtype=np.float32)
    bq = np.asarray(bq, dtype=np.float32)
    bk = np.asarray(bk, dtype=np.float32)
    bv = np.asarray(bv, dtype=np.float32)

    bf = ml_dtypes.bfloat16
    mask = (np.arange(128)[None, :] >= np.arange(128)[:, None]).astype(bf)

    def pmaj(a):  # [D, n] -> [128, NKT, n] partition-major
        return np.ascontiguousarray(a.reshape(NKT, 128, -1).transpose(1, 0, 2))

    in_maps = []
    for c in range(NCORES):
        b, g = divmod(c, 2)
        dsl = slice(g * DIMS, (g + 1) * DIMS)
        in_maps.append(
            {
                "XT": pmaj(np.ascontiguousarray(X[b].T)).astype(bf),
                "WQT": pmaj(np.ascontiguousarray(Wq[dsl, :].T)).astype(bf),
                "WKT": pmaj(np.ascontiguousarray(Wk[dsl, :].T)).astype(bf),
                "WVT": pmaj(np.ascontiguousarray(Wv[dsl, :].T)).astype(bf),
                "BQ": np.ascontiguousarray(bq[dsl].reshape(NDT, 128, 1)),
                "BK": np.ascontiguousarray(bk[dsl].reshape(NDT, 128, 1)),
                "BV": np.ascontiguousarray(bv[dsl].reshape(1, DIMS)).astype(bf),
                "MASKS": mask,
            }
        )
    return in_maps


def _run(in_maps, trace=False, variant=None):
    key = ("nc", variant)
    if key not in _cache:
        kw = dict(VARIANTS.get(variant, {}))
        _cache[key] = _build_kernel(**kw)
    res = run_bass_kernel_spmd(
        _cache[key], in_maps, core_ids=list(range(NCORES)), trace=trace
    )
    return res


VARIANTS = {
    None: {},
    "sps3": {"sps_bufs": 3},
    "po2": {"po_bufs": 2},
    "maskdve": {"masks_on": "vector"},
}


def _assemble(res):
    out = np.empty((B, L, D), dtype=np.float32)
    for c in range(NCORES):
        b, g = divmod(c, 2)
        o = res.results[c]["OUT"]  # [qc, 128, qb, 512]
        out[b, :, g * DIMS : (g + 1) * DIMS] = (
            o.transpose(0, 2, 1, 3).reshape(L, DIMS)
        )
    return out


def kernel(X, Wq, bq, Wk, bk, Wv, bv):
    in_maps = _host_inputs(X, Wq, bq, Wk, bk, Wv, bv)
    res = _run(in_maps, trace=False)
    return _assemble(res)
